# revision 1
# baseline (speedup 1.0000x reference)
"""Trainium2 kernel for nn_Backbone_45320494907484 (dense_transformer).

Sharding: 2-way data-parallel over batch B x 4-way tensor/sequence-parallel
over query rows (mesh (b=2, q=4) over 8 NeuronCores).  Each core owns a
256-query slice of the N^2 attention/bias work for its batch; the residual
stream x is kept replicated within a batch group and re-synchronized with an
all_gather after each attention block.  The small MLP / LN / QKV work is
recomputed redundantly within the group to avoid extra collectives.

Self-contained: shapes/weights layout hardcoded, no sibling imports.
"""

import time
import numpy as np
import jax
import jax.numpy as jnp
from functools import partial
from jax.sharding import Mesh, PartitionSpec as P

try:  # jax>=0.4.35 moved shard_map
    from jax.experimental.shard_map import shard_map
except Exception:  # pragma: no cover
    from jax.shard_map import shard_map

B, N, D, H, L, R = 2, 1024, 128, 8, 8, 4
HD = D // H
DH = int(D * 2.0)
TWO_R = 2 * R

LAST_EXEC_NS = None


def _ln(x, g, b):
    m = x.mean(-1, keepdims=True)
    v = ((x - m) ** 2).mean(-1, keepdims=True)
    return (x - m) / jnp.sqrt(v + 1e-5) * g + b


def _gelu(t):
    return jax.nn.gelu(t, approximate=False)


def _attn_slice(x_full, q0, mask_s, bias_s, qkv_w, proj_w, proj_b, w_pre, w_post):
    """Attention output for 256 query rows [1,256,D]; x_full [1,N,D] (post-LN)."""
    Bx = x_full.shape[0]
    qkv = (x_full @ qkv_w.T).reshape(Bx, N, 3, H, HD).transpose(2, 0, 3, 1, 4)
    q, k, v = qkv[0], qkv[1], qkv[2]          # [1,H,N,hd]
    q_s = jax.lax.dynamic_slice_in_dim(q, q0, 256, axis=2)   # [1,H,256,hd]
    a = jnp.einsum('bhid,bhjd->bhij', q_s, k) * (HD ** -0.5)
    a = a + mask_s[:, None] + bias_s
    a = jnp.einsum('bhij,gh->bgij', a, w_pre)
    a = jax.nn.softmax(a, axis=-1)
    a = jnp.einsum('bhij,gh->bgij', a, w_post)
    o = jnp.einsum('bhij,bhjd->bihd', a, v).reshape(Bx, 256, D)
    return o @ proj_w.T + proj_b


def _block(x, dist_s, am_s, lm_s, et_s, w):
    """Per-shard computation. x [1,N,D] replicated in batch group; *_s are the
    256-row slices of the pairwise tensors for this core."""
    (ln_g, ln_b, qkv_w, proj_w, proj_b, th_pre, th_post, ls, fc1_w, fc1_b,
     fc2_w, fc2_b, de_ln_g, de_ln_b, de_w, de_b, edge_emb, rbf_mu, rbf_sigma) = w

    qidx = jax.lax.axis_index('q')
    q0 = qidx * 256

    rbf = jnp.exp(-(((dist_s[..., None] - rbf_mu) / rbf_sigma) ** 2))  # [1,256,N,R]
    first_mask_s = am_s + lm_s

    def layer(x, l):
        feat = jnp.concatenate([rbf, edge_emb[l][et_s]], axis=-1)      # [1,256,N,2R]
        bias = _gelu(_ln(feat, de_ln_g[l], de_ln_b[l]) @ de_w[l].T + de_b[l])
        bias = jnp.moveaxis(bias, -1, 1)                               # [1,H,256,N]

        for a, mask_s in ((0, first_mask_s), (1, am_s)):
            o_s = _attn_slice(_ln(x, ln_g[l, a], ln_b[l, a]), q0, mask_s, bias,
                              qkv_w[l, a], proj_w[l, a], proj_b[l, a],
                              th_pre[l, a], th_post[l, a])
            o_full = jax.lax.all_gather(o_s, 'q', axis=1, tiled=True)  # [1,N,D]
            x = x + ls[l, a] * o_full

        h = _ln(x, ln_g[l, 2], ln_b[l, 2])
        h = _gelu(h @ fc1_w[l].T + fc1_b[l]) @ fc2_w[l].T + fc2_b[l]
        x = x + ls[l, 2] * h                                           # redundant, replicated
        return x, None

    for l in range(L):
        x, _ = layer(x, l)

    return jax.lax.dynamic_slice_in_dim(x, q0, 256, axis=1)            # [1,256,D]


def kernel(**inputs) -> np.ndarray:
    global LAST_EXEC_NS
    devs = jax.devices()[:8]
    mesh = Mesh(np.array(devs).reshape(2, 4), ('b', 'q'))

    x = jnp.asarray(inputs['x'], jnp.float32)
    dist = jnp.asarray(inputs['dist'], jnp.float32)
    am = jnp.asarray(inputs['attn_mask'], jnp.float32)
    lm = jnp.asarray(inputs['local_mask'], jnp.float32)
    et = jnp.asarray(np.asarray(inputs['edge_type']).astype(np.int32))

    wnames = ['ln_g', 'ln_b', 'qkv_w', 'proj_w', 'proj_b', 'th_pre', 'th_post',
              'ls', 'fc1_w', 'fc1_b', 'fc2_w', 'fc2_b', 'de_ln_g', 'de_ln_b',
              'de_w', 'de_b', 'edge_emb', 'rbf_mu', 'rbf_sigma']
    w = tuple(jnp.asarray(inputs[n], jnp.float32) if n != 'edge_type' else None
              for n in wnames)

    fn = shard_map(_block, mesh=mesh,
                   in_specs=(P('b', None, None), P('b', 'q', None),
                             P('b', 'q', None), P('b', 'q', None),
                             P('b', 'q', None),
                             tuple(P() for _ in w)),
                   out_specs=P('b', 'q', None),
                   check_rep=False)
    fn = jax.jit(fn)

    out = fn(x, dist, am, lm, et, w)
    out.block_until_ready()                    # compile + first run
    t0 = time.perf_counter_ns()
    out = fn(x, dist, am, lm, et, w)
    out.block_until_ready()
    LAST_EXEC_NS = time.perf_counter_ns() - t0

    return np.asarray(out, np.float32)
